# revision 6
# baseline (speedup 1.0000x reference)
"""Causal multi-head attention (B=2, T=2048, D=1024, H=16) on 8 Trainium2 cores.

Sharding: head-parallel, 2 heads per core. Each core computes q/k/v for its
2 heads from the full activations (host pre-transposes x to [D, B*T] so the
contraction dim lands on SBUF partitions), runs causal attention in a
transposed-score layout (S.T = K @ Q.T, so softmax reduction runs on the PE
via a ones-column packed into the V stationary operand), and produces a
partial output projection. The host sums the 8 partials (the w_o
"all-reduce") and concatenates the per-core K/V shards.
"""

import sys

import numpy as np

if "/opt/trn_rl_repo" not in sys.path:
    sys.path.insert(0, "/opt/trn_rl_repo")

B, T, D = 2, 2048, 1024
H, DK = 16, 64
NCORES = 8
HPC = H // NCORES        # heads per core = 2
CPC = HPC * DK           # per-core qkv width = 128
TOK = B * T              # 4096
NQ = 512                 # query chunk
KBS = 128                # key block
NJ = T // NQ             # 4 q-chunks per batch
NKB = T // KBS           # 16 key blocks per batch
SCALE = 1.0 / 8.0        # 1/sqrt(DK)

_PROG = None


def _build():
    import concourse.mybir as mybir
    import concourse.tile as tile
    from concourse import bacc
    from concourse.masks import make_identity

    f32 = mybir.dt.float32
    f32r = mybir.dt.float32r
    Exp = mybir.ActivationFunctionType.Exp

    nc = bacc.Bacc(trn_type="TRN2", target_bir_lowering=False)
    xT_d = nc.declare_dram_parameter("xT", [D, TOK], f32, isOutput=False)
    wT_d = nc.declare_dram_parameter("wT", [D, 3 * CPC], f32, isOutput=False)
    bq_d = nc.declare_dram_parameter("bq", [3, CPC], f32, isOutput=False)
    woT_d = nc.declare_dram_parameter("woT", [CPC, D], f32, isOutput=False)
    msk_d = nc.declare_dram_parameter("msk", [4, KBS, NQ], f32, isOutput=False)
    outp_d = nc.declare_dram_parameter("outp", [TOK, D], f32, isOutput=True)
    kT_d = nc.declare_dram_parameter("kT", [CPC, TOK], f32, isOutput=True)
    vT_d = nc.declare_dram_parameter("vT", [CPC, TOK], f32, isOutput=True)

    def r(ap):
        return ap.bitcast(f32r)

    with tile.TileContext(nc) as tc, nc.allow_low_precision(
        reason="fp32r compute is intentional (12-bit mantissa, within tolerance)"
    ):
        with (
            tc.tile_pool(name="const", bufs=1) as const,
            tc.tile_pool(name="big", bufs=1) as big,
        ):
            wT_sb = const.tile([128, 8, 3 * CPC], f32r, name="wT_sb")
            nc.sync.dma_start(
                wT_sb[:],
                wT_d.ap().rearrange("(c p) f -> p c f", p=128).bitcast(f32r),
            )
            b_sb = const.tile([128, 3], f32, name="b_sb")
            nc.sync.dma_start(b_sb[:], bq_d.ap().rearrange("s p -> p s"))
            woT_sb = const.tile([128, D], f32r, name="woT_sb")
            nc.sync.dma_start(woT_sb[:], woT_d.ap().bitcast(f32r))
            msk_sb = const.tile([128, 4, NQ], f32r, name="msk_sb")
            nc.sync.dma_start(
                msk_sb[:], msk_d.ap().rearrange("r p q -> p r q").bitcast(f32r)
            )
            ident = const.tile([128, 128], f32, name="ident")
            make_identity(nc, ident[:])
            ones_sb = const.tile([65, 64], f32r, name="ones_sb")
            nc.vector.memset(ones_sb[:].bitcast(f32), 1.0)

            qkvT_sb = big.tile([128, 3, TOK], f32r, name="qkvT_sb")
            V_att = big.tile([128, HPC, B, NKB, DK + 1], f32r, name="V_att")
            nc.vector.memset(V_att[:, :, :, :, DK : DK + 1].bitcast(f32), 1.0)
            ctxT_sb = big.tile([128, TOK], f32r, name="ctxT_sb")

            # ---------- Phase 1: qkvT = W_c @ x.T (+bias), V transpose ----------
            with (
                tc.tile_pool(name="xin", bufs=10) as xin,
                tc.tile_pool(name="ps1", bufs=3, space="PSUM") as ps1,
                tc.tile_pool(name="ps1t", bufs=2, space="PSUM") as ps1t,
            ):
                for tb in range(TOK // 512):
                    xts = []
                    for dc in range(8):
                        xt = xin.tile([128, 512], f32r, name="xt", tag="xt")
                        nc.sync.dma_start(
                            xt[:],
                            xT_d[
                                128 * dc : 128 * (dc + 1), 512 * tb : 512 * (tb + 1)
                            ].bitcast(f32r),
                        )
                        xts.append(xt)
                    for sec in range(3):
                        ps = ps1.tile([128, 512], f32, name="ps", tag="qkvps")
                        for dc in range(8):
                            nc.tensor.matmul(
                                ps[:],
                                wT_sb[:, dc, 128 * sec : 128 * (sec + 1)],
                                xts[dc][:],
                                start=(dc == 0),
                                stop=(dc == 7),
                            )
                        dst = qkvT_sb[:, sec, 512 * tb : 512 * (tb + 1)]
                        nc.vector.tensor_scalar_add(dst, ps[:], b_sb[:, sec : sec + 1])
                        if sec == 1:
                            nc.sync.dma_start(
                                kT_d[:, 512 * tb : 512 * (tb + 1)].bitcast(f32r), dst
                            )
                        if sec == 2:
                            nc.sync.dma_start(
                                vT_d[:, 512 * tb : 512 * (tb + 1)].bitcast(f32r), dst
                            )
                            for i in range(4):
                                t0 = 512 * tb + 128 * i
                                bb, kbi = divmod(t0, T)
                                kbi //= KBS
                                tr = ps1t.tile(
                                    [128, 128], f32, name="tr", tag="vtr"
                                )
                                nc.tensor.transpose(
                                    tr[:],
                                    qkvT_sb[:, 2, t0 : t0 + 128].bitcast(f32),
                                    ident[:],
                                )
                                for h in range(HPC):
                                    nc.vector.tensor_copy(
                                        V_att[:, h, bb, kbi, 0:DK],
                                        tr[:, 64 * h : 64 * (h + 1)],
                                    )

            # ---------- Phase 2: causal attention per (batch, q-chunk) ----------
            with (
                tc.tile_pool(name="ps_st", bufs=1, space="PSUM") as ps_st,
                tc.tile_pool(name="ps_cx", bufs=3, space="PSUM") as ps_cx,
                tc.tile_pool(name="ps_bc", bufs=1, space="PSUM") as ps_bc,
                tc.tile_pool(name="ptp", bufs=3) as ptp,
                tc.tile_pool(name="dnp", bufs=4) as dnp,
                tc.tile_pool(name="ctp", bufs=4) as ctp,
            ):
                for b in range(B):
                    for J in range(NJ):
                        q0 = T * b + NQ * J
                        nkb = 4 * J + 4
                        ctxs = []
                        for h in range(HPC):
                            cx = ps_cx.tile([65, 512], f32, name="cx", tag="cx")
                            ctxs.append(cx)
                        for g in range(nkb // 2):
                            st = ps_st.tile([128, 2, 2, 512], f32, name="st", tag="st")
                            for i in range(2):
                                kb = 2 * g + i
                                k0 = T * b + KBS * kb
                                for h in range(HPC):
                                    nc.tensor.matmul(
                                        st[:, i, h, :],
                                        qkvT_sb[64 * h : 64 * (h + 1), 1, k0 : k0 + KBS],
                                        qkvT_sb[64 * h : 64 * (h + 1), 0, q0 : q0 + NQ],
                                        start=True,
                                        stop=True,
                                    )
                            pt = ptp.tile([128, 2, 2, 512], f32r, name="pt", tag="pt")
                            nc.scalar.activation(pt[:], st[:], Exp, scale=SCALE)
                            for i in range(2):
                                kb = 2 * g + i
                                dg = kb - 4 * J
                                if dg >= 0:
                                    for h in range(HPC):
                                        nc.vector.tensor_mul(
                                            pt[:, i, h, :],
                                            pt[:, i, h, :],
                                            msk_sb[:, dg, :],
                                        )
                            for i in range(2):
                                kb = 2 * g + i
                                for h in range(HPC):
                                    nc.tensor.matmul(
                                        ctxs[h][:],
                                        V_att[:, h, b, kb, :],
                                        pt[:, i, h, :],
                                        start=(kb == 0),
                                        stop=(kb == nkb - 1),
                                    )
                        for h in range(HPC):
                            dn = dnp.tile([65, 512], f32r, name="dn", tag="dn")
                            nc.vector.reciprocal(dn[64:65, :], ctxs[h][64:65, :])
                            bc = ps_bc.tile([64, 512], f32, name="bc", tag="bc")
                            nc.tensor.matmul(
                                bc[:],
                                ones_sb[64:65, :],
                                dn[64:65, :],
                                start=True,
                                stop=True,
                            )
                            ct = ctp.tile([64, 512], f32r, name="ct", tag="ct")
                            nc.scalar.copy(ct[:], ctxs[h][0:64, :])
                            nc.vector.tensor_mul(ct[:], ct[:], bc[:])
                            nc.sync.dma_start(
                                ctxT_sb[64 * h : 64 * (h + 1), q0 : q0 + NQ], ct[:]
                            )

            # ---------- Phase 3: partial output projection ----------
            with (
                tc.tile_pool(name="ps3", bufs=4, space="PSUM") as ps3,
                tc.tile_pool(name="osb", bufs=6) as osb,
            ):
                for tb in range(TOK // 128):
                    for of in range(2):
                        po = ps3.tile([128, 512], f32, name="po", tag="po")
                        nc.tensor.matmul(
                            po[:],
                            ctxT_sb[:, 128 * tb : 128 * (tb + 1)],
                            woT_sb[:, 512 * of : 512 * (of + 1)],
                            start=True,
                            stop=True,
                        )
                        ob = osb.tile([128, 512], f32, name="ob", tag="ob")
                        if (tb + of) % 2 == 0:
                            nc.vector.tensor_copy(ob[:], po[:])
                        else:
                            nc.scalar.copy(ob[:], po[:])
                        nc.sync.dma_start(
                            outp_d[
                                128 * tb : 128 * (tb + 1), 512 * of : 512 * (of + 1)
                            ],
                            ob[:],
                        )

    nc.finalize()
    return nc


def _prog():
    global _PROG
    if _PROG is None:
        _PROG = _build()
    return _PROG


def make_in_maps(x, w_qkv, b_qkv, w_o):
    x = np.asarray(x, dtype=np.float32)
    w_qkv = np.asarray(w_qkv, dtype=np.float32)
    b_qkv = np.asarray(b_qkv, dtype=np.float32)
    w_o = np.asarray(w_o, dtype=np.float32)

    xT = np.ascontiguousarray(x.reshape(TOK, D).T)

    kk = np.arange(KBS)[:, None]
    qq = np.arange(NQ)[None, :]
    msk = np.stack(
        [(kk <= qq - KBS * rr).astype(np.float32) for rr in range(4)]
    )  # [4, 128, 512]

    in_maps = []
    for c in range(NCORES):
        sl = slice(c * CPC, (c + 1) * CPC)
        w_c = np.concatenate(
            [w_qkv[sl], w_qkv[D + c * CPC : D + (c + 1) * CPC],
             w_qkv[2 * D + c * CPC : 2 * D + (c + 1) * CPC]]
        )  # [384, 1024]
        wT_c = np.ascontiguousarray(w_c.T)  # [1024, 384]
        bq_c = np.stack(
            [b_qkv[sl], b_qkv[D + c * CPC : D + (c + 1) * CPC],
             b_qkv[2 * D + c * CPC : 2 * D + (c + 1) * CPC]]
        )  # [3, 128]
        woT_c = np.ascontiguousarray(w_o[:, sl].T)  # [128, 1024]
        in_maps.append(
            {"xT": xT, "wT": wT_c, "bq": bq_c, "woT": woT_c, "msk": msk}
        )
    return in_maps


def gather_outputs(results, b_o):
    out = np.zeros((TOK, D), dtype=np.float32)
    k = np.empty((B, H, T, DK), dtype=np.float32)
    v = np.empty((B, H, T, DK), dtype=np.float32)
    for c in range(NCORES):
        out += results[c]["outp"]
        kk = results[c]["kT"].reshape(HPC, DK, B, T).transpose(2, 0, 3, 1)
        vv = results[c]["vT"].reshape(HPC, DK, B, T).transpose(2, 0, 3, 1)
        k[:, HPC * c : HPC * (c + 1)] = kk
        v[:, HPC * c : HPC * (c + 1)] = vv
    out += np.asarray(b_o, dtype=np.float32)[None, :]
    return out.reshape(B, T, D), k, v


def kernel(x, w_qkv, b_qkv, w_o, b_o, _trace=False):
    from concourse.bass_utils import run_bass_kernel_spmd

    nc = _prog()
    in_maps = make_in_maps(x, w_qkv, b_qkv, w_o)
    res = run_bass_kernel_spmd(nc, in_maps, list(range(NCORES)), trace=_trace)
    out, k, v = gather_outputs(res.results, b_o)
    if _trace:
        return (out, k, v), res
    return out, k, v
